# revision 14
# baseline (speedup 1.0000x reference)
"""Category-specific linear layer (MoE-style routing) on 8 Trainium2 cores.

Reference computation:
    out[s] = x[s] @ W[cat_ids[s]] + b[cat_ids[s]]
    x: [64, 256, 1024] f32, cat_ids: [64] int64,
    W: [16, 1024, 4096] f32, b: [16, 4096] f32  ->  out: [64, 256, 4096] f32

Strategy (data-parallel over batch, routing resolved on host):
  - cat_ids is a host-visible input, so the per-sample weight gather is done
    on the host: core c gets samples [8c, 8c+8) plus the 8 matching weight
    matrices, cast to fp16 (PE runs 16-bit matmuls at full rate; fp32 PSUM
    accumulation keeps the error ~1e-3).
  - x is pre-transposed on the host to [K, tokens] so it can serve as the
    stationary matmul operand without an on-chip transpose.
  - One uniform SPMD program for all 8 cores; per-core differences live
    entirely in the input data.
"""

import os
from contextlib import ExitStack

import numpy as np

NCORES = 8
B, T, I, H, C = 64, 256, 1024, 4096, 16
SPC = B // NCORES      # samples per core
TOK = SPC * T          # token rows per core
KT = I // 128          # contraction tiles
NFREE = 512            # matmul moving free dim (one PSUM bank of fp32)
NT = H // NFREE        # n tiles
MT_PER_S = T // 128    # m tiles per sample

_CACHE = {}


def _build_nc(spc=SPC, kt=KT, h=H, nt=NT, mt_per_s=MT_PER_S, nfree=NFREE):
    import concourse.tile as tile
    from concourse import bacc, mybir

    tok = spc * mt_per_s * 128
    nc = bacc.Bacc()
    xT = nc.dram_tensor("xT", [kt, 128, tok], mybir.dt.float16, kind="ExternalInput")
    Wg = nc.dram_tensor("Wg", [spc, kt, 128, h], mybir.dt.float16, kind="ExternalInput")
    out = nc.dram_tensor("out", [tok, h], mybir.dt.float16, kind="ExternalOutput")

    with ExitStack() as ctx:
        tc = ctx.enter_context(tile.TileContext(nc))
        xpool = ctx.enter_context(tc.tile_pool(name="xp", bufs=3 * kt))
        wpool = ctx.enter_context(tc.tile_pool(name="wp", bufs=2 * kt + 2))
        opool = ctx.enter_context(tc.tile_pool(name="op", bufs=2))
        pspool = ctx.enter_context(tc.tile_pool(name="ps", bufs=8, space="PSUM"))

        # Stripe input DMAs across both HWDGE engines (sync + scalar): a
        # single HW queue sustains only ~285 GB/s, below what the weight
        # stream needs to stay ahead of the PE.
        dma_engines = [nc.sync, nc.scalar]
        t_per_s = mt_per_s * 128

        # PE warm-up: ~3.4us of dummy matmuls at kernel start so the HAM
        # clock gate opens while the first weight DMAs are still in flight.
        wupool = ctx.enter_context(tc.tile_pool(name="wu", bufs=1))
        wut = wupool.tile([128, 512], mybir.dt.float16, tag="wu")
        nc.gpsimd.memset(wut[:], 0.0)
        wups = pspool.tile([128, nfree], mybir.dt.float32, tag="ps", name="wups")
        for _ in range(16):
            nc.tensor.matmul(wups[:64, :256], wut[:, :64], wut[:, :256],
                             start=True, stop=True)

        for s in range(spc):
            wtiles = []
            xtiles = []
            for k in range(kt):
                xt_t = xpool.tile([128, t_per_s], mybir.dt.float16, tag="xt")
                dma_engines[k % 2].dma_start(
                    xt_t[:], xT[k, :, s * t_per_s:(s + 1) * t_per_s])
                xtiles.append(xt_t)
                wt = wpool.tile([128, h], mybir.dt.float16, tag="wt")
                dma_engines[k % 2].dma_start(wt[:], Wg[s, k, :, :])
                wtiles.append(wt)
            for mi in range(mt_per_s):
                m = s * mt_per_s + mi
                ot = opool.tile([128, nt * nfree], mybir.dt.float16, tag="ot")
                # n tiles in two groups of nt//2 so PSUM eviction of one
                # group overlaps matmuls of the next (8 banks total).
                gsz = max(nt // 2, 1)
                for g in range(0, nt, gsz):
                    pts = [
                        pspool.tile([128, nfree], mybir.dt.float32, tag="ps",
                                    name=f"ps_{m}_{g}_{j}")
                        for j in range(gsz)
                    ]
                    for k in range(kt):
                        lhsT = xtiles[k][:, mi * 128:(mi + 1) * 128]
                        for j in range(gsz):
                            n = g + j
                            nc.tensor.matmul(
                                pts[j][:],
                                lhsT,
                                wtiles[k][:, n * nfree:(n + 1) * nfree],
                                start=(k == 0),
                                stop=(k == kt - 1),
                            )
                    for j in range(gsz):
                        n = g + j
                        nc.vector.tensor_copy(
                            ot[:, n * nfree:(n + 1) * nfree], pts[j][:]
                        )
                nc.gpsimd.dma_start(out[m * 128:(m + 1) * 128, :], ot[:])
    nc.compile()
    return nc


def _get_nc():
    if "nc" not in _CACHE:
        _CACHE["nc"] = _build_nc()
    return _CACHE["nc"]


def kernel(x, cat_ids, W, b):
    from concourse.bass_utils import run_bass_kernel_spmd

    x = np.asarray(x)
    cat_ids = np.asarray(cat_ids)
    W = np.asarray(W)
    b = np.asarray(b)

    W16 = W.astype(np.float16)
    in_maps = []
    for c in range(NCORES):
        sl = slice(c * SPC, (c + 1) * SPC)
        xs = x[sl].reshape(TOK, I).astype(np.float16)
        xT = np.ascontiguousarray(xs.T).reshape(KT, 128, TOK)
        Wg = np.ascontiguousarray(W16[cat_ids[sl]]).reshape(SPC, KT, 128, H)
        in_maps.append({"xT": xT, "Wg": Wg})

    nc = _get_nc()
    res = run_bass_kernel_spmd(nc, in_maps, core_ids=list(range(NCORES)))
    _CACHE["last_res"] = res

    out = np.empty((B, T, H), dtype=np.float32)
    for c in range(NCORES):
        out[c * SPC:(c + 1) * SPC] = (
            res.results[c]["out"].astype(np.float32).reshape(SPC, T, H)
        )
    if b.any():
        out += b[cat_ids].astype(np.float32)[:, None, :]
    return out


# revision 18
# speedup vs baseline: 1.0246x; 1.0246x over previous
"""Category-specific linear layer (MoE-style routing) on 8 Trainium2 cores.

Reference computation:
    out[s] = x[s] @ W[cat_ids[s]] + b[cat_ids[s]]
    x: [64, 256, 1024] f32, cat_ids: [64] int64,
    W: [16, 1024, 4096] f32, b: [16, 4096] f32  ->  out: [64, 256, 4096] f32

Strategy (data-parallel over batch, routing resolved on host):
  - cat_ids is a host-visible input, so the per-sample weight gather is done
    on the host: core c gets samples [8c, 8c+8) plus the 8 matching weight
    matrices, cast to fp16 (PE runs 16-bit matmuls at full rate; fp32 PSUM
    accumulation keeps the error ~1e-3).
  - x is pre-transposed on the host to [K, tokens] so it can serve as the
    stationary matmul operand without an on-chip transpose.
  - One uniform SPMD program for all 8 cores; per-core differences live
    entirely in the input data.
"""

import os
from contextlib import ExitStack

import numpy as np

NCORES = 8
B, T, I, H, C = 64, 256, 1024, 4096, 16
SPC = B // NCORES      # samples per core
TOK = SPC * T          # token rows per core
KT = I // 128          # contraction tiles
NFREE = 512            # matmul moving free dim (one PSUM bank of fp32)
NT = H // NFREE        # n tiles
MT_PER_S = T // 128    # m tiles per sample

_CACHE = {}


def _build_nc(spc=SPC, kt=KT, h=H, nt=NT, mt_per_s=MT_PER_S, nfree=NFREE):
    import concourse.tile as tile
    from concourse import bacc, mybir

    tok = spc * mt_per_s * 128
    nc = bacc.Bacc()
    xT = nc.dram_tensor("xT", [kt, 128, tok], mybir.dt.float16, kind="ExternalInput")
    Wg = nc.dram_tensor("Wg", [spc, kt, 128, h], mybir.dt.float16, kind="ExternalInput")
    out = nc.dram_tensor("out", [tok, h], mybir.dt.float16, kind="ExternalOutput")

    with ExitStack() as ctx:
        tc = ctx.enter_context(tile.TileContext(nc))
        xpool = ctx.enter_context(tc.tile_pool(name="xp", bufs=kt))
        wpool = ctx.enter_context(tc.tile_pool(name="wp", bufs=2 * kt + 2))
        opool = ctx.enter_context(tc.tile_pool(name="op", bufs=2))
        pspool = ctx.enter_context(tc.tile_pool(name="ps", bufs=8, space="PSUM"))

        # Stripe input DMAs across both HWDGE engines (sync + scalar): a
        # single HW queue sustains only ~285 GB/s, below what the weight
        # stream needs to stay ahead of the PE.
        dma_engines = [nc.sync, nc.scalar]
        t_per_s = mt_per_s * 128

        # PE warm-up: ~3.4us of dummy matmuls at kernel start so the HAM
        # clock gate opens while the first weight DMAs are still in flight.
        wupool = ctx.enter_context(tc.tile_pool(name="wu", bufs=1))
        wut = wupool.tile([128, 512], mybir.dt.float16, tag="wu")
        nc.vector.memset(wut[:], 0.0)
        wups = pspool.tile([128, nfree], mybir.dt.float32, tag="ps", name="wups")
        for _ in range(24):
            nc.tensor.matmul(wups[:64, :256], wut[:, :64], wut[:, :256],
                             start=True, stop=True)

        # xT slabs for the whole core, emitted interleaved with sample 0's
        # weight slabs on the opposite queue so matmul k can start as soon
        # as pair k has landed (instead of after all of xT).
        xtiles = []
        all_wtiles = [[] for _ in range(spc)]
        for k in range(kt):
            xt_t = xpool.tile([128, tok], mybir.dt.float16, tag="xt")
            dma_engines[k % 2].dma_start(xt_t[:], xT[k, :, :])
            xtiles.append(xt_t)
            wt = wpool.tile([128, h], mybir.dt.float16, tag="wt")
            dma_engines[(k + 1) % 2].dma_start(wt[:], Wg[0, k, :, :])
            all_wtiles[0].append(wt)

        for s in range(spc):
            if s > 0:
                for k in range(kt):
                    wt = wpool.tile([128, h], mybir.dt.float16, tag="wt")
                    dma_engines[k % 2].dma_start(wt[:], Wg[s, k, :, :])
                    all_wtiles[s].append(wt)
            wtiles = all_wtiles[s]
            for mi in range(mt_per_s):
                m = s * mt_per_s + mi
                ot = opool.tile([128, nt * nfree], mybir.dt.float16, tag="ot")
                # n tiles in two groups of nt//2 so PSUM eviction of one
                # group overlaps matmuls of the next (8 banks total).
                gsz = max(nt // 2, 1)
                for g in range(0, nt, gsz):
                    pts = [
                        pspool.tile([128, nfree], mybir.dt.float32, tag="ps",
                                    name=f"ps_{m}_{g}_{j}")
                        for j in range(gsz)
                    ]
                    for k in range(kt):
                        lhsT = xtiles[k][:, m * 128:(m + 1) * 128]
                        for j in range(gsz):
                            n = g + j
                            nc.tensor.matmul(
                                pts[j][:],
                                lhsT,
                                wtiles[k][:, n * nfree:(n + 1) * nfree],
                                start=(k == 0),
                                stop=(k == kt - 1),
                            )
                    for j in range(gsz):
                        n = g + j
                        nc.vector.tensor_copy(
                            ot[:, n * nfree:(n + 1) * nfree], pts[j][:]
                        )
                    nc.gpsimd.dma_start(
                        out[m * 128:(m + 1) * 128,
                            g * nfree:(g + gsz) * nfree],
                        ot[:, g * nfree:(g + gsz) * nfree],
                    )
    nc.compile()
    return nc


def _get_nc():
    if "nc" not in _CACHE:
        _CACHE["nc"] = _build_nc()
    return _CACHE["nc"]


def kernel(x, cat_ids, W, b):
    from concourse.bass_utils import run_bass_kernel_spmd

    x = np.asarray(x)
    cat_ids = np.asarray(cat_ids)
    W = np.asarray(W)
    b = np.asarray(b)

    W16 = W.astype(np.float16)
    in_maps = []
    for c in range(NCORES):
        sl = slice(c * SPC, (c + 1) * SPC)
        xs = x[sl].reshape(TOK, I).astype(np.float16)
        xT = np.ascontiguousarray(xs.T).reshape(KT, 128, TOK)
        Wg = np.ascontiguousarray(W16[cat_ids[sl]]).reshape(SPC, KT, 128, H)
        in_maps.append({"xT": xT, "Wg": Wg})

    nc = _get_nc()
    res = run_bass_kernel_spmd(nc, in_maps, core_ids=list(range(NCORES)))
    _CACHE["last_res"] = res

    out = np.empty((B, T, H), dtype=np.float32)
    for c in range(NCORES):
        out[c * SPC:(c + 1) * SPC] = (
            res.results[c]["out"].astype(np.float32).reshape(SPC, T, H)
        )
    if b.any():
        out += b[cat_ids].astype(np.float32)[:, None, :]
    return out


# revision 20
# speedup vs baseline: 1.0782x; 1.0523x over previous
"""Category-specific linear layer (MoE-style routing) on 8 Trainium2 cores.

Reference computation:
    out[s] = x[s] @ W[cat_ids[s]] + b[cat_ids[s]]
    x: [64, 256, 1024] f32, cat_ids: [64] int64,
    W: [16, 1024, 4096] f32, b: [16, 4096] f32  ->  out: [64, 256, 4096] f32

Strategy (data-parallel over batch, routing resolved on host):
  - cat_ids is a host-visible input, so the per-sample weight gather is done
    on the host: core c gets samples [8c, 8c+8) plus the 8 matching weight
    matrices, cast to fp16 (PE runs 16-bit matmuls at full rate; fp32 PSUM
    accumulation keeps the error ~1e-3).
  - x is pre-transposed on the host to [K, tokens] so it can serve as the
    stationary matmul operand without an on-chip transpose.
  - One uniform SPMD program for all 8 cores; per-core differences live
    entirely in the input data.
"""

import os
from contextlib import ExitStack

import numpy as np

NCORES = 8
B, T, I, H, C = 64, 256, 1024, 4096, 16
SPC = B // NCORES      # samples per core
TOK = SPC * T          # token rows per core
KT = I // 128          # contraction tiles
NFREE = 512            # matmul moving free dim (one PSUM bank of fp32)
NT = H // NFREE        # n tiles
MT_PER_S = T // 128    # m tiles per sample

_CACHE = {}


def _build_nc(spc=SPC, kt=KT, h=H, nt=NT, mt_per_s=MT_PER_S, nfree=NFREE):
    import concourse.tile as tile
    from concourse import bacc, mybir

    tok = spc * mt_per_s * 128
    nc = bacc.Bacc()
    xT = nc.dram_tensor("xT", [kt, 128, tok], mybir.dt.float16, kind="ExternalInput")
    Wg = nc.dram_tensor("Wg", [spc, kt, 128, h], mybir.dt.float16, kind="ExternalInput")
    out = nc.dram_tensor("out", [tok, h], mybir.dt.float16, kind="ExternalOutput")

    with ExitStack() as ctx:
        tc = ctx.enter_context(tile.TileContext(nc))
        xpool = ctx.enter_context(tc.tile_pool(name="xp", bufs=kt))
        wpool = ctx.enter_context(tc.tile_pool(name="wp", bufs=2 * kt + 2))
        opool = ctx.enter_context(tc.tile_pool(name="op", bufs=2))
        pspool = ctx.enter_context(tc.tile_pool(name="ps", bufs=8, space="PSUM"))

        # Stripe input DMAs across both HWDGE engines (sync + scalar): a
        # single HW queue sustains only ~285 GB/s, below what the weight
        # stream needs to stay ahead of the PE.
        dma_engines = [nc.sync, nc.scalar]
        t_per_s = mt_per_s * 128

        # PE warm-up: ~3.4us of dummy matmuls at kernel start so the HAM
        # clock gate opens while the first weight DMAs are still in flight.
        wupool = ctx.enter_context(tc.tile_pool(name="wu", bufs=1))
        wut = wupool.tile([128, 512], mybir.dt.float16, tag="wu")
        nc.vector.memset(wut[:], 0.0)
        wups = pspool.tile([128, nfree], mybir.dt.float32, tag="ps", name="wups")
        for _ in range(24):
            nc.tensor.matmul(wups[:64, :256], wut[:, :64], wut[:, :256],
                             start=True, stop=True)

        # xT slabs for the whole core, emitted interleaved with sample 0's
        # weight slabs on the opposite queue so matmul k can start as soon
        # as pair k has landed (instead of after all of xT).
        xtiles = []
        all_wtiles = [[] for _ in range(spc)]
        for k in range(kt):
            xt_t = xpool.tile([128, tok], mybir.dt.float16, tag="xt")
            dma_engines[k % 2].dma_start(xt_t[:], xT[k, :, :])
            xtiles.append(xt_t)
            wt = wpool.tile([128, h], mybir.dt.float16, tag="wt")
            dma_engines[(k + 1) % 2].dma_start(wt[:], Wg[0, k, :, :])
            all_wtiles[0].append(wt)

        for s in range(spc):
            if s > 0:
                for k in range(kt):
                    wt = wpool.tile([128, h], mybir.dt.float16, tag="wt")
                    dma_engines[k % 2].dma_start(wt[:], Wg[s, k, :, :])
                    all_wtiles[s].append(wt)
            wtiles = all_wtiles[s]
            for mi in range(mt_per_s):
                m = s * mt_per_s + mi
                ot = opool.tile([128, nt * nfree], mybir.dt.float16, tag="ot")
                # n tiles in two groups of nt//2 so PSUM eviction of one
                # group overlaps matmuls of the next (8 banks total).
                gsz = max(nt // 2, 1)
                for g in range(0, nt, gsz):
                    pts = [
                        pspool.tile([128, nfree], mybir.dt.float32, tag="ps",
                                    name=f"ps_{m}_{g}_{j}")
                        for j in range(gsz)
                    ]
                    for k in range(kt):
                        lhsT = xtiles[k][:, m * 128:(m + 1) * 128]
                        for j in range(gsz):
                            n = g + j
                            nc.tensor.matmul(
                                pts[j][:],
                                lhsT,
                                wtiles[k][:, n * nfree:(n + 1) * nfree],
                                start=(k == 0),
                                stop=(k == kt - 1),
                            )
                    for j in range(gsz):
                        n = g + j
                        nc.vector.tensor_copy(
                            ot[:, n * nfree:(n + 1) * nfree], pts[j][:]
                        )
                    nc.gpsimd.dma_start(
                        out[m * 128:(m + 1) * 128,
                            g * nfree:(g + gsz) * nfree],
                        ot[:, g * nfree:(g + gsz) * nfree],
                    )
    nc.compile()
    return nc


def _get_nc():
    if "nc" not in _CACHE:
        _CACHE["nc"] = _build_nc()
    return _CACHE["nc"]


def _build_nc_v2(schedules, ncores=NCORES, kt=KT, h=H, nt=NT, mt_per_s=MT_PER_S,
                 nfree=NFREE, spc=SPC):
    """Weight-dedup variant: one SPMD program with a tc.Switch(partition_id)
    dispatch to a per-core schedule.

    schedules[c] = list of m-tile counts per run; run r uses weight slot r.
    Samples are host-sorted by category so each run covers a contiguous
    stretch of m-tiles sharing one weight matrix.
    """
    import concourse.tile as tile
    from concourse import bacc, mybir

    dmax = max(len(s) for s in schedules)
    tok = spc * mt_per_s * 128
    nc = bacc.Bacc()
    xT = nc.dram_tensor("xT", [kt, 128, tok], mybir.dt.float16, kind="ExternalInput")
    Wd = nc.dram_tensor("Wd", [dmax, kt, 128, h], mybir.dt.float16,
                        kind="ExternalInput")
    out = nc.dram_tensor("out", [tok, h], mybir.dt.float16, kind="ExternalOutput")

    with ExitStack() as ctx:
        tc = ctx.enter_context(tile.TileContext(nc))
        xpool = ctx.enter_context(tc.tile_pool(name="xp", bufs=kt))
        wpool = ctx.enter_context(tc.tile_pool(name="wp", bufs=2 * kt))
        opool = ctx.enter_context(tc.tile_pool(name="op", bufs=2))
        pspool = ctx.enter_context(tc.tile_pool(name="ps", bufs=8, space="PSUM"))
        wupool = ctx.enter_context(tc.tile_pool(name="wu", bufs=1))

        dma_engines = [nc.sync, nc.scalar]

        # PE warm-up (opens the HAM clock gate while first DMAs land).
        wut = wupool.tile([128, 512], mybir.dt.float16, tag="wu")
        nc.vector.memset(wut[:], 0.0)
        wups = pspool.tile([128, nfree], mybir.dt.float32, tag="ps", name="wups")
        for _ in range(32):
            nc.tensor.matmul(wups[:64, :128], wut[:, :64], wut[:, :128],
                             start=True, stop=True)

        pid = nc.partition_id()

        for c in tc.Switch(pid, ncores):
            sched = schedules[c]
            # x slabs interleaved with run 0's weight slabs (opposite queues)
            xtiles = []
            run_wtiles = {}
            run_wtiles[0] = []
            for k in range(kt):
                xt_t = xpool.tile([128, tok], mybir.dt.float16, tag="xt",
                                  name=f"xt_{c}_{k}")
                dma_engines[k % 2].dma_start(xt_t[:], xT[k, :, :])
                xtiles.append(xt_t)
                wt = wpool.tile([128, h], mybir.dt.float16, tag="wt",
                                name=f"wt_{c}_0_{k}")
                dma_engines[(k + 1) % 2].dma_start(wt[:], Wd[0, k, :, :])
                run_wtiles[0].append(wt)
            for r in range(1, len(sched)):
                run_wtiles[r] = []
                for k in range(kt):
                    wt = wpool.tile([128, h], mybir.dt.float16, tag="wt",
                                    name=f"wt_{c}_{r}_{k}")
                    dma_engines[k % 2].dma_start(wt[:], Wd[r, k, :, :])
                    run_wtiles[r].append(wt)

            m = 0
            for r, n_mt in enumerate(sched):
                wtiles = run_wtiles[r]
                for _ in range(n_mt):
                    ot = opool.tile([128, nt * nfree], mybir.dt.float16,
                                    tag="ot", name=f"ot_{c}_{m}")
                    gsz = max(nt // 2, 1)
                    for g in range(0, nt, gsz):
                        pts = [
                            pspool.tile([128, nfree], mybir.dt.float32,
                                        tag="ps", name=f"ps_{c}_{m}_{g}_{j}")
                            for j in range(gsz)
                        ]
                        for k in range(kt):
                            lhsT = xtiles[k][:, m * 128:(m + 1) * 128]
                            for j in range(gsz):
                                n = g + j
                                nc.tensor.matmul(
                                    pts[j][:],
                                    lhsT,
                                    wtiles[k][:, n * nfree:(n + 1) * nfree],
                                    start=(k == 0),
                                    stop=(k == kt - 1),
                                )
                        for j in range(gsz):
                            n = g + j
                            nc.vector.tensor_copy(
                                ot[:, n * nfree:(n + 1) * nfree], pts[j][:]
                            )
                        nc.gpsimd.dma_start(
                            out[m * 128:(m + 1) * 128,
                                g * nfree:(g + gsz) * nfree],
                            ot[:, g * nfree:(g + gsz) * nfree],
                        )
                    m += 1
            assert m == spc * mt_per_s, (c, m, sched)
    nc.compile()
    return nc


def kernel(x, cat_ids, W, b):
    from concourse.bass_utils import run_bass_kernel_spmd

    x = np.asarray(x)
    cat_ids = np.asarray(cat_ids).astype(np.int64)
    W = np.asarray(W)
    b = np.asarray(b)

    order = np.argsort(cat_ids, kind="stable")
    W16 = W.astype(np.float16)

    schedules = []
    core_samples = []
    core_cats = []
    for c in range(NCORES):
        samp = order[c * SPC:(c + 1) * SPC]
        cats = []
        counts = []
        for s in samp:
            cid = int(cat_ids[s])
            if not cats or cats[-1] != cid:
                cats.append(cid)
                counts.append(0)
            counts[-1] += MT_PER_S
        schedules.append(tuple(counts))
        core_samples.append(samp)
        core_cats.append(cats)

    sig = tuple(schedules)
    if _CACHE.get("sig") != sig:
        _CACHE["nc2"] = _build_nc_v2([list(s) for s in schedules])
        _CACHE["sig"] = sig
    nc = _CACHE["nc2"]

    dmax = max(len(s) for s in schedules)
    in_maps = []
    for c in range(NCORES):
        samp = core_samples[c]
        xs = x[samp].reshape(TOK, I).astype(np.float16)
        xT = np.ascontiguousarray(xs.T).reshape(KT, 128, TOK)
        cats = list(core_cats[c])
        while len(cats) < dmax:
            cats.append(cats[-1])
        Wd = np.ascontiguousarray(W16[cats]).reshape(dmax, KT, 128, H)
        in_maps.append({"xT": xT, "Wd": Wd})

    res = run_bass_kernel_spmd(nc, in_maps, core_ids=list(range(NCORES)))
    _CACHE["last_res"] = res

    out = np.empty((B, T, H), dtype=np.float32)
    for c in range(NCORES):
        out[core_samples[c]] = (
            res.results[c]["out"].astype(np.float32).reshape(SPC, T, H)
        )
    if b.any():
        out += b[cat_ids].astype(np.float32)[:, None, :]
    return out


# revision 37
# speedup vs baseline: 1.0826x; 1.0040x over previous
"""Category-specific linear layer (MoE-style routing) on 8 Trainium2 cores.

Reference computation:
    out[s] = x[s] @ W[cat_ids[s]] + b[cat_ids[s]]
    x: [64, 256, 1024] f32, cat_ids: [64] int64,
    W: [16, 1024, 4096] f32, b: [16, 4096] f32  ->  out: [64, 256, 4096] f32

Strategy (data-parallel over batch, routing resolved on host):
  - cat_ids is a host-visible input, so the per-sample weight gather is done
    on the host: core c gets samples [8c, 8c+8) plus the 8 matching weight
    matrices, cast to fp16 (PE runs 16-bit matmuls at full rate; fp32 PSUM
    accumulation keeps the error ~1e-3).
  - x is pre-transposed on the host to [K, tokens] so it can serve as the
    stationary matmul operand without an on-chip transpose.
  - One uniform SPMD program for all 8 cores; per-core differences live
    entirely in the input data.
"""

import os
from contextlib import ExitStack

import numpy as np

NCORES = 8
B, T, I, H, C = 64, 256, 1024, 4096, 16
SPC = B // NCORES      # samples per core
TOK = SPC * T          # token rows per core
KT = I // 128          # contraction tiles
NFREE = 512            # matmul moving free dim (one PSUM bank of fp32)
NT = H // NFREE        # n tiles
MT_PER_S = T // 128    # m tiles per sample

_CACHE = {}


def _build_nc(spc=SPC, kt=KT, h=H, nt=NT, mt_per_s=MT_PER_S, nfree=NFREE):
    import concourse.tile as tile
    from concourse import bacc, mybir

    tok = spc * mt_per_s * 128
    nc = bacc.Bacc()
    xT = nc.dram_tensor("xT", [kt, 128, tok], mybir.dt.float16, kind="ExternalInput")
    Wg = nc.dram_tensor("Wg", [spc, kt, 128, h], mybir.dt.float16, kind="ExternalInput")
    out = nc.dram_tensor("out", [tok, h], mybir.dt.float16, kind="ExternalOutput")

    with ExitStack() as ctx:
        tc = ctx.enter_context(tile.TileContext(nc))
        xpool = ctx.enter_context(tc.tile_pool(name="xp", bufs=kt))
        wpool = ctx.enter_context(tc.tile_pool(name="wp", bufs=2 * kt + 2))
        opool = ctx.enter_context(tc.tile_pool(name="op", bufs=2))
        pspool = ctx.enter_context(tc.tile_pool(name="ps", bufs=8, space="PSUM"))

        # Stripe input DMAs across both HWDGE engines (sync + scalar): a
        # single HW queue sustains only ~285 GB/s, below what the weight
        # stream needs to stay ahead of the PE.
        dma_engines = [nc.sync, nc.scalar]
        t_per_s = mt_per_s * 128

        # PE warm-up: ~3.4us of dummy matmuls at kernel start so the HAM
        # clock gate opens while the first weight DMAs are still in flight.
        wupool = ctx.enter_context(tc.tile_pool(name="wu", bufs=1))
        wut = wupool.tile([128, 512], mybir.dt.float16, tag="wu")
        nc.vector.memset(wut[:], 0.0)
        wups = pspool.tile([128, nfree], mybir.dt.float32, tag="ps", name="wups")
        for _ in range(24):
            nc.tensor.matmul(wups[:64, :256], wut[:, :64], wut[:, :256],
                             start=True, stop=True)

        # xT slabs for the whole core, emitted interleaved with sample 0's
        # weight slabs on the opposite queue so matmul k can start as soon
        # as pair k has landed (instead of after all of xT).
        xtiles = []
        all_wtiles = [[] for _ in range(spc)]
        for k in range(kt):
            xt_t = xpool.tile([128, tok], mybir.dt.float16, tag="xt")
            dma_engines[k % 2].dma_start(xt_t[:], xT[k, :, :])
            xtiles.append(xt_t)
            wt = wpool.tile([128, h], mybir.dt.float16, tag="wt")
            dma_engines[(k + 1) % 2].dma_start(wt[:], Wg[0, k, :, :])
            all_wtiles[0].append(wt)

        for s in range(spc):
            if s > 0:
                for k in range(kt):
                    wt = wpool.tile([128, h], mybir.dt.float16, tag="wt")
                    dma_engines[k % 2].dma_start(wt[:], Wg[s, k, :, :])
                    all_wtiles[s].append(wt)
            wtiles = all_wtiles[s]
            for mi in range(mt_per_s):
                m = s * mt_per_s + mi
                ot = opool.tile([128, nt * nfree], mybir.dt.float16, tag="ot")
                # n tiles in two groups of nt//2 so PSUM eviction of one
                # group overlaps matmuls of the next (8 banks total).
                gsz = max(nt // 2, 1)
                for g in range(0, nt, gsz):
                    pts = [
                        pspool.tile([128, nfree], mybir.dt.float32, tag="ps",
                                    name=f"ps_{m}_{g}_{j}")
                        for j in range(gsz)
                    ]
                    for k in range(kt):
                        lhsT = xtiles[k][:, m * 128:(m + 1) * 128]
                        for j in range(gsz):
                            n = g + j
                            nc.tensor.matmul(
                                pts[j][:],
                                lhsT,
                                wtiles[k][:, n * nfree:(n + 1) * nfree],
                                start=(k == 0),
                                stop=(k == kt - 1),
                            )
                    for j in range(gsz):
                        n = g + j
                        nc.vector.tensor_copy(
                            ot[:, n * nfree:(n + 1) * nfree], pts[j][:]
                        )
                    nc.gpsimd.dma_start(
                        out[m * 128:(m + 1) * 128,
                            g * nfree:(g + gsz) * nfree],
                        ot[:, g * nfree:(g + gsz) * nfree],
                    )
    nc.compile()
    return nc


def _get_nc():
    if "nc" not in _CACHE:
        _CACHE["nc"] = _build_nc()
    return _CACHE["nc"]


def _build_nc_v2(schedules, ncores=NCORES, kt=KT, h=H, nt=NT, mt_per_s=MT_PER_S,
                 nfree=NFREE, spc=SPC):
    """Weight-dedup variant: one SPMD program with a tc.Switch(partition_id)
    dispatch to a per-core schedule.

    schedules[c] = list of m-tile counts per run; run r uses weight slot r.
    Samples are host-sorted by category so each run covers a contiguous
    stretch of m-tiles sharing one weight matrix.
    """
    import concourse.tile as tile
    from concourse import bacc, mybir

    dmax = max(len(s) for s in schedules)
    tok = spc * mt_per_s * 128
    nc = bacc.Bacc()
    xT = nc.dram_tensor("xT", [kt, 128, tok], mybir.dt.float16, kind="ExternalInput")
    Wd = nc.dram_tensor("Wd", [dmax, kt, 128, h], mybir.dt.float16,
                        kind="ExternalInput")
    out = nc.dram_tensor("out", [tok, h], mybir.dt.float16, kind="ExternalOutput")

    with ExitStack() as ctx:
        tc = ctx.enter_context(tile.TileContext(nc))
        xpool = ctx.enter_context(tc.tile_pool(name="xp", bufs=kt))
        wpool = ctx.enter_context(tc.tile_pool(name="wp", bufs=2 * kt))
        opool = ctx.enter_context(tc.tile_pool(name="op", bufs=3))
        pspool = ctx.enter_context(tc.tile_pool(name="ps", bufs=8, space="PSUM"))
        wupool = ctx.enter_context(tc.tile_pool(name="wu", bufs=1))

        dma_engines = [nc.sync, nc.scalar]

        # PE warm-up: keep the PE busy from engine boot until the first
        # weight slabs land (~15us) so the HAM clock gate stays open.
        wut = wupool.tile([128, 512], mybir.dt.float16, tag="wu")
        nc.vector.memset(wut[:], 0.0)
        wups = pspool.tile([128, nfree], mybir.dt.float32, tag="ps", name="wups")
        for _ in range(240):
            nc.tensor.matmul(wups[:64, :128], wut[:, :64], wut[:, :128],
                             start=True, stop=True)

        pid = nc.partition_id()

        for c in tc.Switch(pid, ncores):
            sched = schedules[c]
            # x slabs interleaved with run 0's weight slabs on opposite
            # queues so pair k lands together and matmul k starts early.
            xtiles = []
            run_wtiles_c = {0: []}
            for k in range(kt):
                xt_t = xpool.tile([128, tok], mybir.dt.float16, tag="xt",
                                  name=f"xt_{c}_{k}")
                dma_engines[k % 2].dma_start(xt_t[:], xT[k, :, :])
                xtiles.append(xt_t)
                wt = wpool.tile([128, h], mybir.dt.float16, tag="wt",
                                name=f"wt_{c}_0_{k}")
                dma_engines[(k + 1) % 2].dma_start(wt[:], Wd[0, k, :, :])
                run_wtiles_c[0].append(wt)
            for r in range(1, len(sched)):
                run_wtiles_c[r] = []
                for k in range(kt):
                    wt = wpool.tile([128, h], mybir.dt.float16, tag="wt",
                                    name=f"wt_{c}_{r}_{k}")
                    dma_engines[(k + r) % 2].dma_start(wt[:], Wd[r, k, :, :])
                    run_wtiles_c[r].append(wt)
            m = 0
            for r, n_mt in enumerate(sched):
                wtiles = run_wtiles_c[r]
                for _ in range(n_mt):
                    ot = opool.tile([128, nt * nfree], mybir.dt.float16,
                                    tag="ot", name=f"ot_{c}_{m}")
                    gsz = max(nt // 2, 1)
                    for g in range(0, nt, gsz):
                        pts = [
                            pspool.tile([128, nfree], mybir.dt.float32,
                                        tag="ps", name=f"ps_{c}_{m}_{g}_{j}")
                            for j in range(gsz)
                        ]
                        for k in range(kt):
                            lhsT = xtiles[k][:, m * 128:(m + 1) * 128]
                            for j in range(gsz):
                                n = g + j
                                nc.tensor.matmul(
                                    pts[j][:],
                                    lhsT,
                                    wtiles[k][:, n * nfree:(n + 1) * nfree],
                                    start=(k == 0),
                                    stop=(k == kt - 1),
                                )
                        for j in range(gsz):
                            n = g + j
                            nc.vector.tensor_copy(
                                ot[:, n * nfree:(n + 1) * nfree], pts[j][:]
                            )
                        nc.gpsimd.dma_start(
                            out[m * 128:(m + 1) * 128,
                                g * nfree:(g + gsz) * nfree],
                            ot[:, g * nfree:(g + gsz) * nfree],
                        )
                    m += 1
            assert m == spc * mt_per_s, (c, m, sched)
    nc.compile()
    return nc


def _pack_runs(cat_ids):
    """Assign m-tiles (128-token row blocks) to cores, minimizing distinct
    weight matrices per core. Greedy exact-fit-first bin packing over
    per-category m-tile blocks; blocks split across cores when needed.

    Returns per-core (core_mts, core_cats, schedules): core_mts[c] is the
    global m-tile indices this core processes (in run order), core_cats[c]
    the category per run, schedules[c] the m-tile count per run.
    """
    mt_core = SPC * MT_PER_S
    by_cat = {}
    for s, cid in enumerate(cat_ids):
        by_cat.setdefault(int(cid), []).extend(
            range(s * MT_PER_S, (s + 1) * MT_PER_S))
    rem = {k: len(v) for k, v in by_cat.items()}
    used = {k: 0 for k in by_cat}

    core_mts, core_cats, schedules = [], [], []
    for _ in range(NCORES):
        mts, cats, counts = [], [], []
        need = mt_core
        while need > 0:
            avail = [k for k in rem if rem[k] > 0]
            exact = [k for k in avail if rem[k] == need]
            k = exact[0] if exact else max(avail, key=lambda k: rem[k])
            take = min(rem[k], need)
            mts.extend(by_cat[k][used[k]:used[k] + take])
            used[k] += take
            rem[k] -= take
            cats.append(k)
            counts.append(take)
            need -= take
        core_mts.append(np.array(mts))
        core_cats.append(cats)
        schedules.append(tuple(counts))
    assert all(v == 0 for v in rem.values())
    return core_mts, core_cats, schedules


def _kernel_v1(x, cat_ids, W16, b):
    """Per-sample uniform program (no dispatch): robust fallback."""
    from concourse.bass_utils import run_bass_kernel_spmd

    in_maps = []
    for c in range(NCORES):
        sl = slice(c * SPC, (c + 1) * SPC)
        xs = x[sl].reshape(TOK, I).astype(np.float16)
        xT = np.ascontiguousarray(xs.T).reshape(KT, 128, TOK)
        Wg = np.ascontiguousarray(W16[cat_ids[sl]]).reshape(SPC, KT, 128, H)
        in_maps.append({"xT": xT, "Wg": Wg})

    nc = _get_nc()
    res = run_bass_kernel_spmd(nc, in_maps, core_ids=list(range(NCORES)))
    _CACHE["last_res"] = res

    out = np.empty((B, T, H), dtype=np.float32)
    for c in range(NCORES):
        out[c * SPC:(c + 1) * SPC] = (
            res.results[c]["out"].astype(np.float32).reshape(SPC, T, H)
        )
    if b.any():
        out += b[cat_ids].astype(np.float32)[:, None, :]
    return out


def kernel(x, cat_ids, W, b):
    from concourse.bass_utils import run_bass_kernel_spmd

    x = np.asarray(x)
    cat_ids = np.asarray(cat_ids).astype(np.int64)
    W = np.asarray(W)
    b = np.asarray(b)

    W16 = W.astype(np.float16)

    core_mts, core_cats, schedules = _pack_runs(cat_ids)

    sig = tuple(schedules)
    if _CACHE.get("sig") != sig:
        _CACHE["nc2"] = _build_nc_v2([list(s) for s in schedules])
        _CACHE["sig"] = sig
    nc = _CACHE["nc2"]

    dmax = max(len(s) for s in schedules)
    x_rows = x.reshape(B * T, I)
    in_maps = []
    for c in range(NCORES):
        mts = core_mts[c]
        rows = (mts[:, None] * 128 + np.arange(128)[None, :]).reshape(-1)
        xs = x_rows[rows].astype(np.float16)
        xT = np.ascontiguousarray(xs.T).reshape(KT, 128, TOK)
        cats = list(core_cats[c])
        while len(cats) < dmax:
            cats.append(cats[-1])
        Wd = np.ascontiguousarray(W16[cats]).reshape(dmax, KT, 128, H)
        in_maps.append({"xT": xT, "Wd": Wd})

    res = run_bass_kernel_spmd(nc, in_maps, core_ids=list(range(NCORES)))
    _CACHE["last_res"] = res

    out_rows = np.empty((B * T, H), dtype=np.float32)
    for c in range(NCORES):
        mts = core_mts[c]
        rows = (mts[:, None] * 128 + np.arange(128)[None, :]).reshape(-1)
        out_rows[rows] = res.results[c]["out"].astype(np.float32)
    out = out_rows.reshape(B, T, H)
    if b.any():
        out += b[cat_ids].astype(np.float32)[:, None, :]
    return out


# revision 44
# speedup vs baseline: 1.0931x; 1.0097x over previous
"""Category-specific linear layer (MoE-style routing) on 8 Trainium2 cores.

Reference computation:
    out[s] = x[s] @ W[cat_ids[s]] + b[cat_ids[s]]
    x: [64, 256, 1024] f32, cat_ids: [64] int64,
    W: [16, 1024, 4096] f32, b: [16, 4096] f32  ->  out: [64, 256, 4096] f32

Strategy (data-parallel over batch, routing resolved on host):
  - cat_ids is a host-visible input, so the per-sample weight gather is done
    on the host: core c gets samples [8c, 8c+8) plus the 8 matching weight
    matrices, cast to fp16 (PE runs 16-bit matmuls at full rate; fp32 PSUM
    accumulation keeps the error ~1e-3).
  - x is pre-transposed on the host to [K, tokens] so it can serve as the
    stationary matmul operand without an on-chip transpose.
  - One uniform SPMD program for all 8 cores; per-core differences live
    entirely in the input data.
"""

import os
from contextlib import ExitStack

import numpy as np

NCORES = 8
B, T, I, H, C = 64, 256, 1024, 4096, 16
SPC = B // NCORES      # samples per core
TOK = SPC * T          # token rows per core
KT = I // 128          # contraction tiles
NFREE = 512            # matmul moving free dim (one PSUM bank of fp32)
NT = H // NFREE        # n tiles
MT_PER_S = T // 128    # m tiles per sample

_CACHE = {}


def _build_nc(spc=SPC, kt=KT, h=H, nt=NT, mt_per_s=MT_PER_S, nfree=NFREE):
    import concourse.tile as tile
    from concourse import bacc, mybir

    tok = spc * mt_per_s * 128
    nc = bacc.Bacc()
    xT = nc.dram_tensor("xT", [kt, 128, tok], mybir.dt.float16, kind="ExternalInput")
    Wg = nc.dram_tensor("Wg", [spc, kt, 128, h], mybir.dt.float16, kind="ExternalInput")
    out = nc.dram_tensor("out", [tok, h], mybir.dt.float16, kind="ExternalOutput")

    with ExitStack() as ctx:
        tc = ctx.enter_context(tile.TileContext(nc))
        xpool = ctx.enter_context(tc.tile_pool(name="xp", bufs=kt))
        wpool = ctx.enter_context(tc.tile_pool(name="wp", bufs=2 * kt + 2))
        opool = ctx.enter_context(tc.tile_pool(name="op", bufs=2))
        pspool = ctx.enter_context(tc.tile_pool(name="ps", bufs=8, space="PSUM"))

        # Stripe input DMAs across both HWDGE engines (sync + scalar): a
        # single HW queue sustains only ~285 GB/s, below what the weight
        # stream needs to stay ahead of the PE.
        dma_engines = [nc.sync, nc.scalar]
        t_per_s = mt_per_s * 128

        # PE warm-up: ~3.4us of dummy matmuls at kernel start so the HAM
        # clock gate opens while the first weight DMAs are still in flight.
        wupool = ctx.enter_context(tc.tile_pool(name="wu", bufs=1))
        wut = wupool.tile([128, 512], mybir.dt.float16, tag="wu")
        nc.vector.memset(wut[:], 0.0)
        wups = pspool.tile([128, nfree], mybir.dt.float32, tag="ps", name="wups")
        for _ in range(24):
            nc.tensor.matmul(wups[:64, :256], wut[:, :64], wut[:, :256],
                             start=True, stop=True)

        # xT slabs for the whole core, emitted interleaved with sample 0's
        # weight slabs on the opposite queue so matmul k can start as soon
        # as pair k has landed (instead of after all of xT).
        xtiles = []
        all_wtiles = [[] for _ in range(spc)]
        for k in range(kt):
            xt_t = xpool.tile([128, tok], mybir.dt.float16, tag="xt")
            dma_engines[k % 2].dma_start(xt_t[:], xT[k, :, :])
            xtiles.append(xt_t)
            wt = wpool.tile([128, h], mybir.dt.float16, tag="wt")
            dma_engines[(k + 1) % 2].dma_start(wt[:], Wg[0, k, :, :])
            all_wtiles[0].append(wt)

        for s in range(spc):
            if s > 0:
                for k in range(kt):
                    wt = wpool.tile([128, h], mybir.dt.float16, tag="wt")
                    dma_engines[k % 2].dma_start(wt[:], Wg[s, k, :, :])
                    all_wtiles[s].append(wt)
            wtiles = all_wtiles[s]
            for mi in range(mt_per_s):
                m = s * mt_per_s + mi
                ot = opool.tile([128, nt * nfree], mybir.dt.float16, tag="ot")
                # n tiles in two groups of nt//2 so PSUM eviction of one
                # group overlaps matmuls of the next (8 banks total).
                gsz = max(nt // 2, 1)
                for g in range(0, nt, gsz):
                    pts = [
                        pspool.tile([128, nfree], mybir.dt.float32, tag="ps",
                                    name=f"ps_{m}_{g}_{j}")
                        for j in range(gsz)
                    ]
                    for k in range(kt):
                        lhsT = xtiles[k][:, m * 128:(m + 1) * 128]
                        for j in range(gsz):
                            n = g + j
                            nc.tensor.matmul(
                                pts[j][:],
                                lhsT,
                                wtiles[k][:, n * nfree:(n + 1) * nfree],
                                start=(k == 0),
                                stop=(k == kt - 1),
                            )
                    for j in range(gsz):
                        n = g + j
                        nc.vector.tensor_copy(
                            ot[:, n * nfree:(n + 1) * nfree], pts[j][:]
                        )
                    nc.gpsimd.dma_start(
                        out[m * 128:(m + 1) * 128,
                            g * nfree:(g + gsz) * nfree],
                        ot[:, g * nfree:(g + gsz) * nfree],
                    )
    nc.compile()
    return nc


def _get_nc():
    if "nc" not in _CACHE:
        _CACHE["nc"] = _build_nc()
    return _CACHE["nc"]


def _build_nc_v2(schedules, ncores=NCORES, kt=KT, h=H, nt=NT, mt_per_s=MT_PER_S,
                 nfree=NFREE, spc=SPC):
    """Weight-dedup variant: one SPMD program with a tc.Switch(partition_id)
    dispatch to a per-core schedule.

    schedules[c] = list of m-tile counts per run; run r uses weight slot r.
    Samples are host-sorted by category so each run covers a contiguous
    stretch of m-tiles sharing one weight matrix.
    """
    import concourse.tile as tile
    from concourse import bacc, mybir

    dmax = max(len(s) for s in schedules)
    tok = spc * mt_per_s * 128
    nc = bacc.Bacc()
    xT = nc.dram_tensor("xT", [kt, 128, tok], mybir.dt.float16, kind="ExternalInput")
    Wd = nc.dram_tensor("Wd", [dmax, kt, 128, h], mybir.dt.float16,
                        kind="ExternalInput")
    out = nc.dram_tensor("out", [tok, h], mybir.dt.float16, kind="ExternalOutput")

    with ExitStack() as ctx:
        tc = ctx.enter_context(tile.TileContext(nc))
        xpool = ctx.enter_context(tc.tile_pool(name="xp", bufs=2 * kt))
        wpool = ctx.enter_context(tc.tile_pool(name="wp", bufs=2 * kt + 2))
        opool = ctx.enter_context(tc.tile_pool(name="op", bufs=3))
        pspool = ctx.enter_context(tc.tile_pool(name="ps", bufs=8, space="PSUM"))
        wupool = ctx.enter_context(tc.tile_pool(name="wu", bufs=1))

        dma_engines = [nc.sync, nc.scalar]

        # PE warm-up: keep the PE busy from engine boot until the first
        # weight slabs land (~15us) so the HAM clock gate stays open.
        wut = wupool.tile([128, 512], mybir.dt.float16, tag="wu")
        nc.vector.memset(wut[:], 0.0)
        wups = pspool.tile([128, nfree], mybir.dt.float32, tag="ps", name="wups")
        for _ in range(60):
            nc.tensor.matmul(wups[:64, :128], wut[:, :64], wut[:, :128],
                             start=True, stop=True)

        pid = nc.partition_id()

        half = tok // 2
        mt_half = half // 128

        for c in tc.Switch(pid, ncores):
            sched = schedules[c]
            # Startup order: first x halves (m-tiles 0..7) paired with run
            # 0's weight slabs on opposite queues, so m-tile 0 can start
            # trickle-computing as slab pairs land; second x halves and
            # later runs follow.
            xtiles = [[None, None] for _ in range(kt)]
            run_wtiles_c = {0: []}
            for k in range(kt):
                xt_t = xpool.tile([128, half], mybir.dt.float16, tag="xt",
                                  name=f"xta_{c}_{k}")
                dma_engines[k % 2].dma_start(xt_t[:], xT[k, :, :half])
                xtiles[k][0] = xt_t
                wt = wpool.tile([128, h], mybir.dt.float16, tag="wt",
                                name=f"wt_{c}_0_{k}")
                dma_engines[(k + 1) % 2].dma_start(wt[:], Wd[0, k, :, :])
                run_wtiles_c[0].append(wt)
            for k in range(kt):
                xt_t = xpool.tile([128, half], mybir.dt.float16, tag="xt",
                                  name=f"xtb_{c}_{k}")
                dma_engines[k % 2].dma_start(xt_t[:], xT[k, :, half:])
                xtiles[k][1] = xt_t
            for r in range(1, len(sched)):
                run_wtiles_c[r] = []
                for k in range(kt):
                    wt = wpool.tile([128, h], mybir.dt.float16, tag="wt",
                                    name=f"wt_{c}_{r}_{k}")
                    dma_engines[(k + r) % 2].dma_start(wt[:], Wd[r, k, :, :])
                    run_wtiles_c[r].append(wt)
            m = 0
            for r, n_mt in enumerate(sched):
                wtiles = run_wtiles_c[r]
                for _ in range(n_mt):
                    ot = opool.tile([128, nt * nfree], mybir.dt.float16,
                                    tag="ot", name=f"ot_{c}_{m}")
                    gsz = max(nt // 2, 1)
                    for g in range(0, nt, gsz):
                        pts = [
                            pspool.tile([128, nfree], mybir.dt.float32,
                                        tag="ps", name=f"ps_{c}_{m}_{g}_{j}")
                            for j in range(gsz)
                        ]
                        mh, mo = divmod(m, mt_half)
                        for k in range(kt):
                            lhsT = xtiles[k][mh][:, mo * 128:(mo + 1) * 128]
                            for j in range(gsz):
                                n = g + j
                                nc.tensor.matmul(
                                    pts[j][:],
                                    lhsT,
                                    wtiles[k][:, n * nfree:(n + 1) * nfree],
                                    start=(k == 0),
                                    stop=(k == kt - 1),
                                )
                        for j in range(gsz):
                            n = g + j
                            nc.vector.tensor_copy(
                                ot[:, n * nfree:(n + 1) * nfree], pts[j][:]
                            )
                        nc.gpsimd.dma_start(
                            out[m * 128:(m + 1) * 128,
                                g * nfree:(g + gsz) * nfree],
                            ot[:, g * nfree:(g + gsz) * nfree],
                        )
                    m += 1
            assert m == spc * mt_per_s, (c, m, sched)
    nc.compile()
    return nc


def _pack_runs(cat_ids):
    """Assign m-tiles (128-token row blocks) to cores, minimizing distinct
    weight matrices per core. Greedy exact-fit-first bin packing over
    per-category m-tile blocks; blocks split across cores when needed.

    Returns per-core (core_mts, core_cats, schedules): core_mts[c] is the
    global m-tile indices this core processes (in run order), core_cats[c]
    the category per run, schedules[c] the m-tile count per run.
    """
    mt_core = SPC * MT_PER_S
    by_cat = {}
    for s, cid in enumerate(cat_ids):
        by_cat.setdefault(int(cid), []).extend(
            range(s * MT_PER_S, (s + 1) * MT_PER_S))
    rem = {k: len(v) for k, v in by_cat.items()}
    used = {k: 0 for k in by_cat}

    core_mts, core_cats, schedules = [], [], []
    for _ in range(NCORES):
        mts, cats, counts = [], [], []
        need = mt_core
        while need > 0:
            avail = [k for k in rem if rem[k] > 0]
            exact = [k for k in avail if rem[k] == need]
            k = exact[0] if exact else max(avail, key=lambda k: rem[k])
            take = min(rem[k], need)
            mts.extend(by_cat[k][used[k]:used[k] + take])
            used[k] += take
            rem[k] -= take
            cats.append(k)
            counts.append(take)
            need -= take
        core_mts.append(np.array(mts))
        core_cats.append(cats)
        schedules.append(tuple(counts))
    assert all(v == 0 for v in rem.values())
    return core_mts, core_cats, schedules


def _kernel_v1(x, cat_ids, W16, b):
    """Per-sample uniform program (no dispatch): robust fallback."""
    from concourse.bass_utils import run_bass_kernel_spmd

    in_maps = []
    for c in range(NCORES):
        sl = slice(c * SPC, (c + 1) * SPC)
        xs = x[sl].reshape(TOK, I).astype(np.float16)
        xT = np.ascontiguousarray(xs.T).reshape(KT, 128, TOK)
        Wg = np.ascontiguousarray(W16[cat_ids[sl]]).reshape(SPC, KT, 128, H)
        in_maps.append({"xT": xT, "Wg": Wg})

    nc = _get_nc()
    res = run_bass_kernel_spmd(nc, in_maps, core_ids=list(range(NCORES)))
    _CACHE["last_res"] = res

    out = np.empty((B, T, H), dtype=np.float32)
    for c in range(NCORES):
        out[c * SPC:(c + 1) * SPC] = (
            res.results[c]["out"].astype(np.float32).reshape(SPC, T, H)
        )
    if b.any():
        out += b[cat_ids].astype(np.float32)[:, None, :]
    return out


def kernel(x, cat_ids, W, b):
    from concourse.bass_utils import run_bass_kernel_spmd

    x = np.asarray(x)
    cat_ids = np.asarray(cat_ids).astype(np.int64)
    W = np.asarray(W)
    b = np.asarray(b)

    W16 = W.astype(np.float16)

    core_mts, core_cats, schedules = _pack_runs(cat_ids)

    sig = tuple(schedules)
    if _CACHE.get("sig") != sig:
        _CACHE["nc2"] = _build_nc_v2([list(s) for s in schedules])
        _CACHE["sig"] = sig
    nc = _CACHE["nc2"]

    dmax = max(len(s) for s in schedules)
    x_rows = x.reshape(B * T, I)
    in_maps = []
    for c in range(NCORES):
        mts = core_mts[c]
        rows = (mts[:, None] * 128 + np.arange(128)[None, :]).reshape(-1)
        xs = x_rows[rows].astype(np.float16)
        xT = np.ascontiguousarray(xs.T).reshape(KT, 128, TOK)
        cats = list(core_cats[c])
        while len(cats) < dmax:
            cats.append(cats[-1])
        Wd = np.ascontiguousarray(W16[cats]).reshape(dmax, KT, 128, H)
        in_maps.append({"xT": xT, "Wd": Wd})

    res = run_bass_kernel_spmd(nc, in_maps, core_ids=list(range(NCORES)))
    _CACHE["last_res"] = res

    out_rows = np.empty((B * T, H), dtype=np.float32)
    for c in range(NCORES):
        mts = core_mts[c]
        rows = (mts[:, None] * 128 + np.arange(128)[None, :]).reshape(-1)
        out_rows[rows] = res.results[c]["out"].astype(np.float32)
    out = out_rows.reshape(B, T, H)
    if b.any():
        out += b[cat_ids].astype(np.float32)[:, None, :]
    return out


# revision 47
# speedup vs baseline: 1.1198x; 1.0244x over previous
"""Category-specific linear layer (MoE-style routing) on 8 Trainium2 cores.

Reference computation:
    out[s] = x[s] @ W[cat_ids[s]] + b[cat_ids[s]]
    x: [64, 256, 1024] f32, cat_ids: [64] int64,
    W: [16, 1024, 4096] f32, b: [16, 4096] f32  ->  out: [64, 256, 4096] f32

Strategy (data-parallel over batch, routing resolved on host):
  - cat_ids is a host-visible input, so the per-sample weight gather is done
    on the host: core c gets samples [8c, 8c+8) plus the 8 matching weight
    matrices, cast to fp16 (PE runs 16-bit matmuls at full rate; fp32 PSUM
    accumulation keeps the error ~1e-3).
  - x is pre-transposed on the host to [K, tokens] so it can serve as the
    stationary matmul operand without an on-chip transpose.
  - One uniform SPMD program for all 8 cores; per-core differences live
    entirely in the input data.
"""

import os
from contextlib import ExitStack

import numpy as np

NCORES = 8
B, T, I, H, C = 64, 256, 1024, 4096, 16
SPC = B // NCORES      # samples per core
TOK = SPC * T          # token rows per core
KT = I // 128          # contraction tiles
NFREE = 512            # matmul moving free dim (one PSUM bank of fp32)
NT = H // NFREE        # n tiles
MT_PER_S = T // 128    # m tiles per sample

_CACHE = {}


def _build_nc(spc=SPC, kt=KT, h=H, nt=NT, mt_per_s=MT_PER_S, nfree=NFREE):
    import concourse.tile as tile
    from concourse import bacc, mybir

    tok = spc * mt_per_s * 128
    nc = bacc.Bacc()
    xT = nc.dram_tensor("xT", [kt, 128, tok], mybir.dt.float16, kind="ExternalInput")
    Wg = nc.dram_tensor("Wg", [spc, kt, 128, h], mybir.dt.float16, kind="ExternalInput")
    out = nc.dram_tensor("out", [tok, h], mybir.dt.float16, kind="ExternalOutput")

    with ExitStack() as ctx:
        tc = ctx.enter_context(tile.TileContext(nc))
        xpool = ctx.enter_context(tc.tile_pool(name="xp", bufs=kt))
        wpool = ctx.enter_context(tc.tile_pool(name="wp", bufs=2 * kt + 2))
        opool = ctx.enter_context(tc.tile_pool(name="op", bufs=2))
        pspool = ctx.enter_context(tc.tile_pool(name="ps", bufs=8, space="PSUM"))

        # Stripe input DMAs across both HWDGE engines (sync + scalar): a
        # single HW queue sustains only ~285 GB/s, below what the weight
        # stream needs to stay ahead of the PE.
        dma_engines = [nc.sync, nc.scalar]
        t_per_s = mt_per_s * 128

        # PE warm-up: ~3.4us of dummy matmuls at kernel start so the HAM
        # clock gate opens while the first weight DMAs are still in flight.
        wupool = ctx.enter_context(tc.tile_pool(name="wu", bufs=1))
        wut = wupool.tile([128, 512], mybir.dt.float16, tag="wu")
        nc.vector.memset(wut[:], 0.0)
        wups = pspool.tile([128, nfree], mybir.dt.float32, tag="ps", name="wups")
        for _ in range(24):
            nc.tensor.matmul(wups[:64, :256], wut[:, :64], wut[:, :256],
                             start=True, stop=True)

        # xT slabs for the whole core, emitted interleaved with sample 0's
        # weight slabs on the opposite queue so matmul k can start as soon
        # as pair k has landed (instead of after all of xT).
        xtiles = []
        all_wtiles = [[] for _ in range(spc)]
        for k in range(kt):
            xt_t = xpool.tile([128, tok], mybir.dt.float16, tag="xt")
            dma_engines[k % 2].dma_start(xt_t[:], xT[k, :, :])
            xtiles.append(xt_t)
            wt = wpool.tile([128, h], mybir.dt.float16, tag="wt")
            dma_engines[(k + 1) % 2].dma_start(wt[:], Wg[0, k, :, :])
            all_wtiles[0].append(wt)

        for s in range(spc):
            if s > 0:
                for k in range(kt):
                    wt = wpool.tile([128, h], mybir.dt.float16, tag="wt")
                    dma_engines[k % 2].dma_start(wt[:], Wg[s, k, :, :])
                    all_wtiles[s].append(wt)
            wtiles = all_wtiles[s]
            for mi in range(mt_per_s):
                m = s * mt_per_s + mi
                ot = opool.tile([128, nt * nfree], mybir.dt.float16, tag="ot")
                # n tiles in two groups of nt//2 so PSUM eviction of one
                # group overlaps matmuls of the next (8 banks total).
                gsz = max(nt // 2, 1)
                for g in range(0, nt, gsz):
                    pts = [
                        pspool.tile([128, nfree], mybir.dt.float32, tag="ps",
                                    name=f"ps_{m}_{g}_{j}")
                        for j in range(gsz)
                    ]
                    for k in range(kt):
                        lhsT = xtiles[k][:, m * 128:(m + 1) * 128]
                        for j in range(gsz):
                            n = g + j
                            nc.tensor.matmul(
                                pts[j][:],
                                lhsT,
                                wtiles[k][:, n * nfree:(n + 1) * nfree],
                                start=(k == 0),
                                stop=(k == kt - 1),
                            )
                    for j in range(gsz):
                        n = g + j
                        nc.vector.tensor_copy(
                            ot[:, n * nfree:(n + 1) * nfree], pts[j][:]
                        )
                    nc.gpsimd.dma_start(
                        out[m * 128:(m + 1) * 128,
                            g * nfree:(g + gsz) * nfree],
                        ot[:, g * nfree:(g + gsz) * nfree],
                    )
    nc.compile()
    return nc


def _get_nc():
    if "nc" not in _CACHE:
        _CACHE["nc"] = _build_nc()
    return _CACHE["nc"]


def _build_nc_v2(schedules, ncores=NCORES, kt=KT, h=H, nt=NT, mt_per_s=MT_PER_S,
                 nfree=NFREE, spc=SPC):
    """Weight-dedup variant: one SPMD program with a tc.Switch(partition_id)
    dispatch to a per-core schedule.

    schedules[c] = list of m-tile counts per run; run r uses weight slot r.
    Samples are host-sorted by category so each run covers a contiguous
    stretch of m-tiles sharing one weight matrix.
    """
    import concourse.tile as tile
    from concourse import bacc, mybir

    dmax = max(len(s) for s in schedules)
    tok = spc * mt_per_s * 128
    nc = bacc.Bacc()
    xT = nc.dram_tensor("xT", [kt, 128, tok], mybir.dt.float16, kind="ExternalInput")
    Wd = nc.dram_tensor("Wd", [dmax, kt, 128, h], mybir.dt.float16,
                        kind="ExternalInput")
    out = nc.dram_tensor("out", [tok, h], mybir.dt.float16, kind="ExternalOutput")

    with ExitStack() as ctx:
        tc = ctx.enter_context(tile.TileContext(nc))
        xpool = ctx.enter_context(tc.tile_pool(name="xp", bufs=2 * kt))
        wpool = ctx.enter_context(tc.tile_pool(name="wp", bufs=2 * kt + 2))
        opool = ctx.enter_context(tc.tile_pool(name="op", bufs=3))
        pspool = ctx.enter_context(tc.tile_pool(name="ps", bufs=8, space="PSUM"))
        wupool = ctx.enter_context(tc.tile_pool(name="wu", bufs=1))

        dma_engines = [nc.sync, nc.scalar]

        # PE warm-up: keep the PE busy from engine boot until the first
        # weight slabs land (~15us) so the HAM clock gate stays open.
        wut = wupool.tile([128, 512], mybir.dt.float16, tag="wu")
        nc.vector.memset(wut[:], 0.0)
        wups = pspool.tile([128, nfree], mybir.dt.float32, tag="ps", name="wups")
        for _ in range(60):
            nc.tensor.matmul(wups[:64, :128], wut[:, :64], wut[:, :128],
                             start=True, stop=True)

        pid = nc.partition_id()

        half = tok // 2
        mt_half = half // 128

        for c in tc.Switch(pid, ncores):
            sched = schedules[c]
            # Startup order: first x halves (m-tiles 0..7) paired with run
            # 0's weight slabs on opposite queues, so m-tile 0 can start
            # trickle-computing as slab pairs land; second x halves and
            # later runs follow.
            xtiles = [[None, None] for _ in range(kt)]
            run_wtiles_c = {0: []}
            for k in range(kt):
                xt_t = xpool.tile([128, half], mybir.dt.float16, tag="xt",
                                  name=f"xta_{c}_{k}")
                dma_engines[k % 2].dma_start(xt_t[:], xT[k, :, :half])
                xtiles[k][0] = xt_t
                wt = wpool.tile([128, h], mybir.dt.float16, tag="wt",
                                name=f"wt_{c}_0_{k}")
                dma_engines[(k + 1) % 2].dma_start(wt[:], Wd[0, k, :, :])
                run_wtiles_c[0].append(wt)
            for k in range(kt):
                xt_t = xpool.tile([128, half], mybir.dt.float16, tag="xt",
                                  name=f"xtb_{c}_{k}")
                dma_engines[k % 2].dma_start(xt_t[:], xT[k, :, half:])
                xtiles[k][1] = xt_t
            for r in range(1, len(sched)):
                run_wtiles_c[r] = []
                for k in range(kt):
                    wt = wpool.tile([128, h], mybir.dt.float16, tag="wt",
                                    name=f"wt_{c}_{r}_{k}")
                    dma_engines[(k + r) % 2].dma_start(wt[:], Wd[r, k, :, :])
                    run_wtiles_c[r].append(wt)
            # First two m-tiles of run 0, interleaved across the 8 PSUM
            # banks (4 each): during the startup trickle each arriving
            # weight slab enables ~3.6us of matmul work (two m-tiles' worth)
            # instead of 1.8us, denser than the ~2.9us slab arrival rate —
            # the PE never idles long enough for HAM to re-throttle.
            w0 = run_wtiles_c[0]
            gsz = max(nt // 2, 1)
            ots01 = [opool.tile([128, nt * nfree], mybir.dt.float16,
                                tag="ot", name=f"ot_{c}_{mp}")
                     for mp in range(2)]
            for g in range(0, nt, gsz):
                pts2 = [
                    [pspool.tile([128, nfree], mybir.dt.float32, tag="ps",
                                 name=f"ps_{c}_{mp}_{g}_{j}")
                     for j in range(gsz)]
                    for mp in range(2)
                ]
                for k in range(kt):
                    for mp in range(2):
                        lhsT = xtiles[k][0][:, mp * 128:(mp + 1) * 128]
                        for j in range(gsz):
                            n = g + j
                            nc.tensor.matmul(
                                pts2[mp][j][:],
                                lhsT,
                                w0[k][:, n * nfree:(n + 1) * nfree],
                                start=(k == 0),
                                stop=(k == kt - 1),
                            )
                for mp in range(2):
                    for j in range(gsz):
                        n = g + j
                        nc.vector.tensor_copy(
                            ots01[mp][:, n * nfree:(n + 1) * nfree],
                            pts2[mp][j][:],
                        )
                    nc.gpsimd.dma_start(
                        out[mp * 128:(mp + 1) * 128,
                            g * nfree:(g + gsz) * nfree],
                        ots01[mp][:, g * nfree:(g + gsz) * nfree],
                    )

            m = 2
            sched_rest = [(0, sched[0] - 2)] + [
                (r, n_mt) for r, n_mt in enumerate(sched) if r > 0]
            for r, n_mt in sched_rest:
                wtiles = run_wtiles_c[r]
                for _ in range(n_mt):
                    ot = opool.tile([128, nt * nfree], mybir.dt.float16,
                                    tag="ot", name=f"ot_{c}_{m}")
                    gsz = max(nt // 2, 1)
                    for g in range(0, nt, gsz):
                        pts = [
                            pspool.tile([128, nfree], mybir.dt.float32,
                                        tag="ps", name=f"ps_{c}_{m}_{g}_{j}")
                            for j in range(gsz)
                        ]
                        mh, mo = divmod(m, mt_half)
                        for k in range(kt):
                            lhsT = xtiles[k][mh][:, mo * 128:(mo + 1) * 128]
                            for j in range(gsz):
                                n = g + j
                                nc.tensor.matmul(
                                    pts[j][:],
                                    lhsT,
                                    wtiles[k][:, n * nfree:(n + 1) * nfree],
                                    start=(k == 0),
                                    stop=(k == kt - 1),
                                )
                        for j in range(gsz):
                            n = g + j
                            nc.vector.tensor_copy(
                                ot[:, n * nfree:(n + 1) * nfree], pts[j][:]
                            )
                        nc.gpsimd.dma_start(
                            out[m * 128:(m + 1) * 128,
                                g * nfree:(g + gsz) * nfree],
                            ot[:, g * nfree:(g + gsz) * nfree],
                        )
                    m += 1
            assert m == spc * mt_per_s, (c, m, sched)
    nc.compile()
    return nc


def _pack_runs(cat_ids):
    """Assign m-tiles (128-token row blocks) to cores, minimizing distinct
    weight matrices per core. Greedy exact-fit-first bin packing over
    per-category m-tile blocks; blocks split across cores when needed.

    Returns per-core (core_mts, core_cats, schedules): core_mts[c] is the
    global m-tile indices this core processes (in run order), core_cats[c]
    the category per run, schedules[c] the m-tile count per run.
    """
    mt_core = SPC * MT_PER_S
    by_cat = {}
    for s, cid in enumerate(cat_ids):
        by_cat.setdefault(int(cid), []).extend(
            range(s * MT_PER_S, (s + 1) * MT_PER_S))
    rem = {k: len(v) for k, v in by_cat.items()}
    used = {k: 0 for k in by_cat}

    core_mts, core_cats, schedules = [], [], []
    for _ in range(NCORES):
        mts, cats, counts = [], [], []
        need = mt_core
        while need > 0:
            avail = [k for k in rem if rem[k] > 0]
            exact = [k for k in avail if rem[k] == need]
            k = exact[0] if exact else max(avail, key=lambda k: rem[k])
            take = min(rem[k], need)
            mts.extend(by_cat[k][used[k]:used[k] + take])
            used[k] += take
            rem[k] -= take
            cats.append(k)
            counts.append(take)
            need -= take
        core_mts.append(np.array(mts))
        core_cats.append(cats)
        schedules.append(tuple(counts))
    assert all(v == 0 for v in rem.values())
    return core_mts, core_cats, schedules


def _kernel_v1(x, cat_ids, W16, b):
    """Per-sample uniform program (no dispatch): robust fallback."""
    from concourse.bass_utils import run_bass_kernel_spmd

    in_maps = []
    for c in range(NCORES):
        sl = slice(c * SPC, (c + 1) * SPC)
        xs = x[sl].reshape(TOK, I).astype(np.float16)
        xT = np.ascontiguousarray(xs.T).reshape(KT, 128, TOK)
        Wg = np.ascontiguousarray(W16[cat_ids[sl]]).reshape(SPC, KT, 128, H)
        in_maps.append({"xT": xT, "Wg": Wg})

    nc = _get_nc()
    res = run_bass_kernel_spmd(nc, in_maps, core_ids=list(range(NCORES)))
    _CACHE["last_res"] = res

    out = np.empty((B, T, H), dtype=np.float32)
    for c in range(NCORES):
        out[c * SPC:(c + 1) * SPC] = (
            res.results[c]["out"].astype(np.float32).reshape(SPC, T, H)
        )
    if b.any():
        out += b[cat_ids].astype(np.float32)[:, None, :]
    return out


def kernel(x, cat_ids, W, b):
    from concourse.bass_utils import run_bass_kernel_spmd

    x = np.asarray(x)
    cat_ids = np.asarray(cat_ids).astype(np.int64)
    W = np.asarray(W)
    b = np.asarray(b)

    W16 = W.astype(np.float16)

    core_mts, core_cats, schedules = _pack_runs(cat_ids)

    sig = tuple(schedules)
    if _CACHE.get("sig") != sig:
        _CACHE["nc2"] = _build_nc_v2([list(s) for s in schedules])
        _CACHE["sig"] = sig
    nc = _CACHE["nc2"]

    dmax = max(len(s) for s in schedules)
    x_rows = x.reshape(B * T, I)
    in_maps = []
    for c in range(NCORES):
        mts = core_mts[c]
        rows = (mts[:, None] * 128 + np.arange(128)[None, :]).reshape(-1)
        xs = x_rows[rows].astype(np.float16)
        xT = np.ascontiguousarray(xs.T).reshape(KT, 128, TOK)
        cats = list(core_cats[c])
        while len(cats) < dmax:
            cats.append(cats[-1])
        Wd = np.ascontiguousarray(W16[cats]).reshape(dmax, KT, 128, H)
        in_maps.append({"xT": xT, "Wd": Wd})

    res = run_bass_kernel_spmd(nc, in_maps, core_ids=list(range(NCORES)))
    _CACHE["last_res"] = res

    out_rows = np.empty((B * T, H), dtype=np.float32)
    for c in range(NCORES):
        mts = core_mts[c]
        rows = (mts[:, None] * 128 + np.arange(128)[None, :]).reshape(-1)
        out_rows[rows] = res.results[c]["out"].astype(np.float32)
    out = out_rows.reshape(B, T, H)
    if b.any():
        out += b[cat_ids].astype(np.float32)[:, None, :]
    return out
